# revision 12
# baseline (speedup 1.0000x reference)
"""Two-layer GAT (GraphAttention) forward on 8 Trainium2 NeuronCores.

Math (per layer, reference semantics):
    h  = x @ w                      [N, U]
    a1 = h @ aw1 ; a2 = h @ aw2     [N, H]
    P[i,j,h]    = exp(adj[i,j] * relu(a1[i,h] + a2[j,h]))
    attn[i,j,h] = P / sum_j P
    out[i,h,:]  = sum_j attn[i,j,h] * h[j,:]   -> concat heads -> activation

Key identity used here: with adj in {0,1},
    P[i,j] = max(adj[i,j] * e1[i] * e2[j], 1),   e1 = exp(a1), e2 = exp(a2)
and softmax rows are invariant to scaling by em1[i] = exp(-a1[i]):
    P'[j,i] = max(adjT[j,i] * e2[j], em1[i])
so per j-tile the work is one scalar-by-partition multiply (B = adjT * e2,
split between ScalarE and VectorE) and one grouped tensor_tensor max on
VectorE against a broadcast em1 row; numerator and denominator both come
out of a single PE matmul against [1 | h] extended features.

Layer-0 head outputs are staged as a [128, R] bf16 block (17 rows per head
at 32-aligned partition offsets: [denom | h*16]); two block matmuls then
produce all 4 numerators and 4 denominators at once, one reciprocal +
multiply + a ones-matmul row-sum give the layer-1 feature hfe.

Sharding: rows (i) of the score matrix are partitioned 512 per core;
adj rows are fed pre-transposed per core as [N, 512] (j on partitions).
All small weights are replicated; hfe of layer 1 is all-gathered.
"""

import sys

for _p in ("/opt/trn_rl_repo",):
    if _p not in sys.path:
        sys.path.insert(0, _p)

from contextlib import ExitStack

import ml_dtypes
import numpy as np

import concourse.bacc as bacc
import concourse.mybir as mybir
import concourse.tile as tile
from concourse.bass_utils import run_bass_kernel_spmd

F32 = mybir.dt.float32
BF16 = mybir.dt.bfloat16
BF = ml_dtypes.bfloat16

N = 4096          # nodes
FIN = 128         # input features
U0 = 16           # layer-0 units
H0 = 4            # layer-0 heads
NCORES = 8
R = N // NCORES   # local rows per core (512)
NJT = N // 128    # j tiles (32)
GRP = 8           # j-tiles fused per tensor_tensor max
# Engine per B tile within each group: 's'=ScalarE, 'v'=VectorE (which
# also runs the grouped max, so it gets the smaller share).
ASSIGN = ['s', 's', 's', 's', 'v', 'v', 'v', 'v']
NGRP = NJT // GRP
U1 = U0 + 1       # 17: [denom | h] rows per head

_CACHE = {}


def _build():
    nc = bacc.Bacc("TRN2", target_bir_lowering=False, debug=False,
                   num_devices=NCORES)

    # ---- I/O ----
    d_adjT = nc.dram_tensor("adjT", [N, R], BF16, kind="ExternalInput")
    d_xT = nc.dram_tensor("xT", [FIN, N], BF16, kind="ExternalInput")
    d_xTl = nc.dram_tensor("xTl", [FIN, R], BF16, kind="ExternalInput")
    d_prep = nc.dram_tensor("prep", [FIN, U0 + H0], BF16, kind="ExternalInput")
    d_v1 = nc.dram_tensor("v1", [FIN, H0], BF16, kind="ExternalInput")
    d_w1blk = nc.dram_tensor("w1blk", [128, 2 * H0], BF16,
                             kind="ExternalInput")
    d_consts = nc.dram_tensor("consts", [4, 2], BF16, kind="ExternalInput")
    d_ident = nc.dram_tensor("ident", [32, 32], BF16, kind="ExternalInput")
    d_aw11 = nc.dram_tensor("aw11", [1, 1], F32, kind="ExternalInput")
    d_aw21 = nc.dram_tensor("aw21", [1, 1], F32, kind="ExternalInput")
    d_y = nc.dram_tensor("y", [1, R], F32, kind="ExternalOutput")

    with ExitStack() as ctx:
        tc = ctx.enter_context(tile.TileContext(nc))
        const = ctx.enter_context(tc.tile_pool(name="const", bufs=1))
        work = ctx.enter_context(tc.tile_pool(name="work", bufs=1))
        bpool = ctx.enter_context(tc.tile_pool(name="bpool", bufs=3))
        ppool = ctx.enter_context(tc.tile_pool(name="ppool", bufs=3))
        accs = ctx.enter_context(tc.tile_pool(name="accs", bufs=2))
        dram = ctx.enter_context(tc.tile_pool(name="dram", bufs=1, space="DRAM"))
        pp_misc = ctx.enter_context(tc.tile_pool(name="pp_misc", bufs=2, space="PSUM"))
        pp_hj = ctx.enter_context(tc.tile_pool(name="pp_hj", bufs=2, space="PSUM"))
        pp_acc = ctx.enter_context(tc.tile_pool(name="pp_acc", bufs=2, space="PSUM"))

        # ---- persistent SBUF ----
        sb_adjT = const.tile([128, NJT * R], BF16, tag="adjT")     # 32KB/p
        sb_xT = const.tile([FIN, N], BF16, tag="xT")               # 8KB/p
        sb_xTl = const.tile([FIN, R], BF16, tag="xTl")
        sb_prep = const.tile([FIN, U0 + H0], BF16, tag="prep")     # [w0 | v2]
        sb_v1 = const.tile([FIN, H0], BF16, tag="v1")
        sb_w1blk = const.tile([128, 2 * H0], BF16, tag="w1blk")
        sb_consts = const.tile([4, 2], BF16, tag="consts")
        sb_ident = const.tile([32, 32], BF16, tag="ident")
        sb_aw11 = const.tile([1, 1], F32, tag="aw11")
        sb_naw11 = const.tile([1, 1], F32, tag="naw11")
        sb_aw21bc = const.tile([128, 1], F32, tag="aw21bc")
        sb_hj = const.tile([128, NJT * U1], BF16, tag="hj")        # [1 | h] per jt
        sb_e2j = const.tile([128, NJT * H0], F32, tag="e2j")
        sb_em1bc = [const.tile([128, R], BF16, tag=f"em1bc{h}",
                       name=f"em1bc{h}") for h in range(H0)]
        sb_em1s = [const.tile([1, R], BF16, tag=f"em1s{h}", name=f"em1s{h}")
                   for h in range(H0)]
        sb_h1b = const.tile([128, R], BF16, tag="h1b")  # heads at 32h..32h+16
        sb_d4 = const.tile([H0, R], F32, tag="d4")
        sb_rec4 = const.tile([H0, R], F32, tag="rec4")
        sb_hfp4 = const.tile([H0, R], BF16, tag="hfp4")
        sb_hfeT = const.tile([1, R], BF16, tag="hfeT")
        sb_p012 = const.tile([1, R], BF16, tag="p012")
        sb_em11 = const.tile([1, R], BF16, tag="em11")
        sb_em1bc1 = const.tile([128, R], BF16, tag="em1bc1")
        sb_g32 = const.tile([32, 128], BF16, tag="g32")
        sb_e2j1 = const.tile([128, NJT], F32, tag="e2j1")
        sb_hfe1e = const.tile([128, NJT * 2], BF16, tag="hfe1e")
        sb_sigd = work.tile([1, 1], F32, tag="sigd")
        sb_l1f = work.tile([2, R], F32, tag="l1f")
        sb_l1b = work.tile([2, R], BF16, tag="l1b")
        sb_d1 = work.tile([1, R], F32, tag="d1")
        sb_fin = work.tile([1, R], F32, tag="fin")
        sb_fin2 = work.tile([1, R], F32, tag="fin2")

        d_em1 = dram.tile([H0, R], BF16)  # per-head rows
        d_em11 = dram.tile([1, R], BF16)
        d_gin = dram.tile([1, R], BF16)
        d_gout = dram.tile([NCORES, R], BF16, addr_space="Shared")

        # ---- load constants / inputs ----
        def load_adjT(m):
            src = d_adjT[256 * m:256 * (m + 1), :].rearrange(
                "(g p) i -> p g i", p=128)
            dst = sb_adjT[:, 1024 * m:1024 * (m + 1)].rearrange(
                "p (g i) -> p g i", g=2)
            nc.sync.dma_start(dst, src)

        def load_xT(xc):
            nc.sync.dma_start(sb_xT[:, 1024 * xc:1024 * (xc + 1)],
                              d_xT[:, 1024 * xc:1024 * (xc + 1)])

        def em1_head(h):
            ps_a1 = pp_misc.tile([1, R], F32, tag="misc", name="ps_a1")
            nc.tensor.matmul(ps_a1[:], sb_v1[:, h:h + 1], sb_xTl[:],
                             start=True, stop=True)
            nc.scalar.activation(sb_em1s[h][:], ps_a1[:],
                                 mybir.ActivationFunctionType.Exp, scale=-1.0)
            nc.gpsimd.dma_start(d_em1[h:h + 1, :], sb_em1s[h][:])
            nc.gpsimd.dma_start(sb_em1bc[h][:],
                              d_em1[h:h + 1, :].to_broadcast((128, R)))

        nc.sync.dma_start(sb_xTl[:], d_xTl[:])
        nc.sync.dma_start(sb_v1[:], d_v1[:])
        nc.sync.dma_start(sb_prep[:], d_prep[:])
        load_xT(0)
        for m in range(4):
            load_adjT(m)
        nc.sync.dma_start(sb_w1blk[:], d_w1blk[:])
        nc.sync.dma_start(sb_consts[:], d_consts[:])
        nc.sync.dma_start(sb_ident[:], d_ident[:])
        nc.sync.dma_start(sb_aw11[:], d_aw11[:])
        nc.sync.dma_start(sb_aw21bc[:], d_aw21[0:1, 0:1].to_broadcast((128, 1)))

        nc.vector.memset(sb_h1b[:], 0.0)
        em1_head(0)

        # ---- prep: h/e2 per j-tile (prefetched ahead of head-0 groups) ----
        nc.vector.memset(sb_hj[:], 1.0)
        W = U0 + H0

        def prep_chunk(q4):
            ps4 = pp_hj.tile([128, 4 * W], F32, tag="hj", name="ps4")
            for q in range(4):
                jt = 4 * q4 + q
                nc.tensor.matmul(ps4[:, W * q:W * (q + 1)],
                                 sb_xT[:, 128 * jt:128 * (jt + 1)],
                                 sb_prep[:], start=True, stop=True)
            hjv = sb_hj[:, 4 * U1 * q4:4 * U1 * (q4 + 1)].rearrange(
                "p (q c) -> p q c", q=4)[:, :, 1:U0 + 1]
            psv = ps4[:].rearrange("p (q c) -> p q c", q=4)[:, :, 0:U0]
            nc.scalar.copy(hjv, psv)
            e2v = sb_e2j[:, 4 * H0 * q4:4 * H0 * (q4 + 1)].rearrange(
                "p (q c) -> p q c", q=4)
            pse = ps4[:].rearrange("p (q c) -> p q c", q=4)[:, :, U0:U0 + H0]
            nc.scalar.activation(e2v, pse, mybir.ActivationFunctionType.Exp)

        nc.vector.memset(sb_hfe1e[:], 1.0)
        prep_chunk(0)
        prep_chunk(1)

        def make_group(jts, e2_ap, em1bc, ps_acc, lhsT_of):
            """B tiles on ScalarE/VectorE, one grouped max, matmuls."""
            t_B = bpool.tile([128, GRP * R], BF16, tag="B")
            order = [k for k in range(GRP) if ASSIGN[k] == 's'] + \
                    [k for k in range(GRP) if ASSIGN[k] == 'v']
            for k in order:
                jt = jts[k]
                dst = t_B[:, R * k:R * (k + 1)]
                src = sb_adjT[:, R * jt:R * (jt + 1)]
                if ASSIGN[k] == 's':
                    nc.scalar.mul(dst, src, e2_ap(jt))
                else:
                    nc.vector.tensor_single_scalar(dst, src, e2_ap(jt),
                                                   mybir.AluOpType.mult)
            t_P = ppool.tile([128, GRP * R], BF16, tag="P")
            nc.vector.tensor_tensor(
                t_P[:].rearrange("p (g i) -> p g i", g=GRP),
                t_B[:].rearrange("p (g i) -> p g i", g=GRP),
                em1bc[:, None, :].to_broadcast((128, GRP, R)),
                mybir.AluOpType.max)
            for k in range(GRP):
                jt = jts[k]
                nc.tensor.matmul(ps_acc, lhsT_of(jt), t_P[:, R * k:R * (k + 1)],
                                 start=(jt == 0), stop=(jt == NJT - 1))

        # ---- layer 0 main ----
        for h in range(H0):
            ps_acc = pp_acc.tile([U1, R], F32, tag="acc")
            for g in range(NGRP):
                if h == 0:
                    if g < 3:
                        em1_head(g + 1)
                        load_xT(g + 1)
                        for m in range(4 * g + 4, 4 * g + 8):
                            load_adjT(m)
                    for c in range(2 * (g + 1), min(2 * (g + 2), NJT // 4)):
                        prep_chunk(c)
                if h == 3 and g == 1:
                    # heads 0-2 projection overlaps head-3 compute
                    ps_n = pp_misc.tile([3, R], F32, tag="misc", name="ps_n")
                    ps_d = pp_misc.tile([3, R], F32, tag="misc", name="ps_d")
                    nc.tensor.matmul(ps_n[:], sb_w1blk[:, 0:3], sb_h1b[:],
                                     start=True, stop=True)
                    nc.tensor.matmul(ps_d[:], sb_w1blk[:, H0:H0 + 3],
                                     sb_h1b[:], start=True, stop=True)
                    nc.scalar.copy(sb_d4[0:3, :], ps_d[:])
                    nc.vector.reciprocal_approx_accurate(
                        sb_rec4[0:3, :], sb_d4[0:3, :],
                        accs.tile([3, R], F32, tag="rscr", name="rscr"))
                    nc.vector.tensor_mul(sb_hfp4[0:3, :], ps_n[:],
                                         sb_rec4[0:3, :])
                    ps_p012 = pp_misc.tile([1, R], F32, tag="misc",
                                           name="ps_p012")
                    nc.tensor.matmul(ps_p012[:], sb_consts[0:3, 0:1],
                                     sb_hfp4[0:3, :], start=True, stop=True)
                    nc.scalar.copy(sb_p012[:], ps_p012[:])
                make_group(
                    [GRP * g + k for k in range(GRP)],
                    lambda jt: sb_e2j[:, H0 * jt + h:H0 * jt + h + 1],
                    sb_em1bc[h], ps_acc[:],
                    lambda jt: sb_hj[:, U1 * jt:U1 * (jt + 1)])
            nc.scalar.activation(sb_h1b[32 * h:32 * h + U1, :], ps_acc[:],
                                 mybir.ActivationFunctionType.Relu)

        # ---- layer 1 prep: hfe = p012 + w1_3.h1_3 / denom_3 ----
        ps_n3 = pp_misc.tile([1, R], F32, tag="misc", name="ps_n3")
        ps_d3 = pp_misc.tile([1, R], F32, tag="misc", name="ps_d3")
        nc.tensor.matmul(ps_n3[:], sb_w1blk[:, 3:4], sb_h1b[:],
                         start=True, stop=True)
        nc.tensor.matmul(ps_d3[:], sb_w1blk[:, H0 + 3:H0 + 4], sb_h1b[:],
                         start=True, stop=True)
        nc.scalar.copy(sb_d4[0:1, :], ps_d3[:])
        nc.vector.reciprocal_approx_accurate(
            sb_rec4[0:1, :], sb_d4[0:1, :],
            accs.tile([1, R], F32, tag="rscr3", name="rscr3"))
        nc.vector.tensor_mul(sb_hfp4[0:1, :], ps_n3[:], sb_rec4[0:1, :])
        nc.vector.tensor_add(sb_hfeT[:], sb_p012[:], sb_hfp4[0:1, :])
        nc.gpsimd.dma_start(d_gin[:], sb_hfeT[:])
        nc.scalar.mul(sb_naw11[:], sb_aw11[:], -1.0)
        nc.scalar.activation(sb_em11[:], sb_hfeT[:],
                             mybir.ActivationFunctionType.Exp,
                             scale=sb_naw11[:])
        nc.sync.dma_start(d_em11[:], sb_em11[:])
        nc.sync.dma_start(sb_em1bc1[:], d_em11[0:1, :].to_broadcast((128, R)))

        nc.gpsimd.collective_compute(
            "AllGather", mybir.AluOpType.bypass,
            replica_groups=[list(range(NCORES))],
            ins=[d_gin[:].opt()], outs=[d_gout[:].opt()])
        gflat = d_gout[:].rearrange("a b -> (a b)").rearrange(
            "(t p) -> t p", p=128)
        nc.gpsimd.dma_start(sb_g32[:], gflat)
        ps_hf = pp_misc.tile([128, NJT], BF16, tag="misc", name="ps_hf")
        nc.tensor.transpose(ps_hf[:], sb_g32[:], sb_ident[:])
        nc.scalar.activation(sb_e2j1[:], ps_hf[:],
                             mybir.ActivationFunctionType.Exp,
                             scale=sb_aw21bc[:])
        # prefetch the sigmoid activation table after the last exp
        nc.scalar.activation(sb_sigd[:], sb_sigd[:],
                             mybir.ActivationFunctionType.Sigmoid)
        nc.vector.tensor_copy(
            sb_hfe1e[:].rearrange("p (t two) -> p t two", two=2)[:, :, 0:1],
            ps_hf[:][:, :, None])

        # ---- layer 1 main: numer+denom from one [128,2] lhsT matmul ----
        ps_l1 = pp_acc.tile([2, R], F32, tag="accl1", name="ps_l1", bufs=1)
        for g in range(NGRP):
            make_group(
                [GRP * g + k for k in range(GRP)],
                lambda jt: sb_e2j1[:, jt:jt + 1],
                sb_em1bc1, ps_l1[:],
                lambda jt: sb_hfe1e[:, 2 * jt:2 * jt + 2])

        # ---- final: sigmoid(numer/denom) ----
        nc.scalar.copy(sb_l1f[:], ps_l1[:])
        nc.vector.tensor_copy(sb_l1b[:], sb_l1f[:])
        ps_d1 = pp_misc.tile([1, R], F32, tag="misc", name="ps_d1")
        nc.tensor.matmul(ps_d1[:], sb_consts[0:2, 1:2], sb_l1b[:],
                         start=True, stop=True)
        nc.scalar.copy(sb_d1[:], ps_d1[:])
        sb_fscr = accs.tile([1, R], F32, tag="fscr", name="sb_fscr")
        nc.vector.reciprocal_approx_accurate(sb_fin[:], sb_d1[:], sb_fscr[:])
        nc.vector.tensor_mul(sb_fin2[:], sb_l1f[0:1, :], sb_fin[:])
        nc.scalar.activation(sb_fin[:], sb_fin2[:],
                             mybir.ActivationFunctionType.Sigmoid)
        nc.sync.dma_start(d_y[:], sb_fin[:])

    nc.compile()
    return nc


def _prep_inputs(x, adj, w0, aw1_0, aw2_0, w1, aw1_1, aw2_1):
    x = np.asarray(x, np.float32)
    adj = np.asarray(adj, np.float32)
    xT = np.ascontiguousarray(x.T.astype(BF))
    adjT = np.asarray(adj.T, BF)                        # [N, N], exact 0/1
    w0f = np.asarray(w0, np.float32)
    v1 = np.ascontiguousarray((w0f @ np.asarray(aw1_0, np.float32)).astype(BF))
    v2 = (w0f @ np.asarray(aw2_0, np.float32)).astype(BF)
    prep = np.ascontiguousarray(
        np.concatenate([w0f.astype(BF), v2], axis=1))
    w1f = np.asarray(w1, np.float32).reshape(H0, U0)
    # block matrix over the 32-aligned stacked head rows: cols 0..3 pick
    # numerators (w1 per head), cols 4..7 pick the denominator rows.
    w1blk = np.zeros((128, 2 * H0), np.float32)
    for h in range(H0):
        w1blk[32 * h + 1:32 * h + U1, h] = w1f[h]
        w1blk[32 * h, H0 + h] = 1.0
    w1blk = np.ascontiguousarray(w1blk.astype(BF))
    consts = np.zeros((4, 2), np.float32)
    consts[:, 0] = 1.0     # ones4 for the head-sum matmul
    consts[1, 1] = 1.0     # e01 selector for the layer-1 denominator row
    consts = np.ascontiguousarray(consts.astype(BF))
    ident = np.ascontiguousarray(np.eye(32, dtype=np.float32).astype(BF))
    aw11 = np.asarray(aw1_1, np.float32).reshape(1, 1)
    aw21 = np.asarray(aw2_1, np.float32).reshape(1, 1)
    in_maps = []
    for c in range(NCORES):
        rows = slice(R * c, R * (c + 1))
        in_maps.append({
            "adjT": np.ascontiguousarray(adjT[:, rows]),
            "xT": xT,
            "xTl": np.ascontiguousarray(xT[:, rows]),
            "prep": prep, "v1": v1, "w1blk": w1blk, "consts": consts,
            "ident": ident,
            "aw11": aw11, "aw21": aw21,
        })
    return in_maps


def run(inputs, trace=False):
    if "nc" not in _CACHE:
        _CACHE["nc"] = _build()
    nc = _CACHE["nc"]
    in_maps = _prep_inputs(**inputs)
    res = run_bass_kernel_spmd(nc, in_maps, list(range(NCORES)), trace=trace)
    y = np.concatenate([res.results[c]["y"][0] for c in range(NCORES)])
    return np.ascontiguousarray(y.astype(np.float32)), res


def kernel(**inputs):
    y, _ = run(inputs)
    return y
